# revision 7
# baseline (speedup 1.0000x reference)
"""BranchRoute (2-branch threshold MoE routing) Trainium2 kernel.

Full-input contract: kernel(x, gate_w, gate_b) -> (x0, x1, combined),
x: [8192, 4096] f32, gate_w: [4096, 2] f32, gate_b: [2] f32.

Math: z = x @ gate_w + gate_b; m_i = z_i > 0  (== sigmoid(z_i) > 0.5);
x0 = x * m0, x1 = x * m1, combined = x * (m0 + m1).

Sharding: data-parallel over tokens, 8 shards of 1024 tokens, one per
NeuronCore; gate weights replicated; no cross-core communication.

Precision: the gate (z and the masks) is computed entirely in f32 and
is bit-identical to the f32 baseline, so routing decisions are exact.
The three dense outputs are stored to HBM as bf16 and widened to f32 on
the host (exact 16-bit left shift). That cuts per-core HBM traffic from
64 MiB (f32 in + 3x f32 out) to 40 MiB (f32 in + 3x bf16 out); the
per-element bf16 rounding gives a norm relative error ~1.1e-3.

Raw Bass (no Tile: the local walrus build encodes at most ONE sem wait
per instruction, while Tile attaches multi-waits). Four engine
programs with explicit semaphores, one wait per wait-instruction.
DMA completions are not ordered across transfers, so every DMA
semaphore tracks at most one outstanding transfer (per-buffer-slot
sems) and waits are unambiguous.

Per 128-token tile (b = it%2 out slot, xs = it%3 in slot):
  DVE:  prod0 = x*w0 (PSUM), prod1 = x*w1, m0 = z0 > -b0, m1 = z1 > -b1,
        oc = o0 + o1 (bf16+bf16, 2x mode).
  ACT:  z0 = reduce(prod0), z1 = reduce(prod1) (Copy with accum_out),
        o0 = Copy(x, scale=m0) -> bf16; all x loads (ACT HWDGE ring).
  Pool: o1 = x * m1 -> bf16 (gpsimd); x1 stores + odd-tile combined
        stores (SWDGE ring).
  SP:   x0 stores + even-tile combined stores; weight/bias broadcast.

Engine busy/tile ~11.5 us (DVE) / ~11.6 (ACT) / ~10 (Pool) vs DMA
~14.6 us -> DMA-bound. Memory target: 40 MiB/core at ~360 GB/s
~ 117 us steady state.
"""

import sys

import numpy as np

sys.path.insert(0, "/opt/trn_rl_repo")

import concourse.bass as bass
from concourse import mybir
from concourse.bass_utils import run_bass_kernel_spmd

N_CORES = 8
N, D = 8192, 4096
SHARD = N // N_CORES  # 1024 tokens per core
P = 128
NT = SHARD // P  # 8 tiles per core
F32 = mybir.dt.float32
BF16 = mybir.dt.bfloat16
Copy = mybir.ActivationFunctionType.Copy
Alu = mybir.AluOpType

_CACHE = {}


def _build(nt=NT, n_pass=1):
    nc = bass.Bass()
    x_in = nc.dram_tensor("x", [SHARD, D], F32, kind="ExternalInput")
    gw_in = nc.dram_tensor("gate_w", [D, 2], F32, kind="ExternalInput")
    gb_in = nc.dram_tensor("gate_b", [2], F32, kind="ExternalInput")
    x0_out = nc.dram_tensor("x0", [SHARD, D], BF16, kind="ExternalOutput")
    x1_out = nc.dram_tensor("x1", [SHARD, D], BF16, kind="ExternalOutput")
    xc_out = nc.dram_tensor("combined", [SHARD, D], BF16, kind="ExternalOutput")

    NPT = nt * n_pass  # total tile iterations (n_pass > 1: timing loops)

    def tid(it):  # tile row index within the shard for iteration it
        return it % nt

    from contextlib import ExitStack

    with ExitStack() as ctx:
        sb = lambda name, shape, dt=F32: ctx.enter_context(
            nc.sbuf_tensor(name, list(shape), dt)
        )
        sem = lambda name: ctx.enter_context(nc.semaphore(name))
        gwb = sb("gwb", (P, 2 * D))  # interleaved w0/w1 bcast
        bb = sb("bb", (P, 2))  # bias bcast
        nb = sb("nb", (P, 2))  # -bias
        xt0 = sb("xt0", (P, D))
        xt1 = sb("xt1", (P, D))
        xt2 = sb("xt2", (P, D))
        prod0 = ctx.enter_context(nc.psum_tensor("prod0", [P, D], F32))
        prod1 = sb("prod1", (P, D))
        z = sb("z", (P, 2))
        m = sb("m", (P, 2))
        o0a = sb("o0a", (P, D), BF16)
        o0b = sb("o0b", (P, D), BF16)
        o1a = sb("o1a", (P, D), BF16)
        o1b = sb("o1b", (P, D), BF16)
        oca = sb("oca", (P, D), BF16)
        ocb = sb("ocb", (P, D), BF16)
        setup_sem = sem("setup_sem")
        inx0 = sem("inx0")
        inx1 = sem("inx1")
        inx2 = sem("inx2")
        so0a = sem("so0a")
        so0b = sem("so0b")
        so1a = sem("so1a")
        so1b = sem("so1b")
        soca = sem("soca")
        socb = sem("socb")
        vec_sem = sem("vec_sem")
        act_sem = sem("act_sem")
        pool_sem = sem("pool_sem")
        block = ctx.enter_context(nc.Block())
        xt = [xt0, xt1, xt2]
        o0 = [o0a, o0b]
        o1 = [o1a, o1b]
        oc = [oca, ocb]
        inx = [inx0, inx1, inx2]
        so0 = [so0a, so0b]
        so1 = [so1a, so1b]
        soc = [soca, socb]
        # de-interleaved strided views of the broadcast weights [P, D]
        gw_v = gwb[:].rearrange("p (d t) -> p t d", t=2)
        w0v = gw_v[:, 0:1, :].rearrange("p one d -> p (one d)")
        w1v = gw_v[:, 1:2, :].rearrange("p one d -> p (one d)")

        # semaphore bookkeeping:
        #   setup_sem: gwb + bb loads -> 32
        #   inx[s]: x loads for slot s (3 slots); load(it) completes at
        #     16*(it//3+1); all loads on the ACT HWDGE ring.
        #   so0/so1/soc[b]: output stores per slot; store(it) completes
        #     at 16*(it//2+1); slot free for tile it when >= 16*(it//2)
        #   vec_sem: setup nb op = 1; then 5 ops/tile -> 1+5*it+k, k=1..5
        #     (1: mult0, 2: mult1, 3: m0, 4: m1, 5: oc add)
        #   act_sem: 3 ops/tile -> 3*it+k, k=1..3 (1: z0, 2: z1, 3: o0)
        #   pool_sem: 1 op/tile -> it+1 (o1)
        V = lambda it, k: 1 + 5 * it + k
        A = lambda it, k: 3 * it + k

        def x_done(it):  # x-load completions for slot it%3 up to tile it
            return 16 * (it // 3 + 1)

        def slot_free(it):  # store-slot completions freeing slot it%2
            return 16 * (it // 2)

        def slot_done(it):  # store completions up to and incl tile it
            return 16 * (it // 2 + 1)

        @block.sync
        def _(sync):
            gw_flat = gw_in[:, :].rearrange("d t -> (d t)")
            sync.dma_start(
                gwb[:],
                bass.AP(gw_flat.tensor, gw_flat.offset, [[0, P], [1, 2 * D]]),
            ).then_inc(setup_sem, 16)
            gb_flat = gb_in[:]
            sync.dma_start(
                bb[:], bass.AP(gb_flat.tensor, gb_flat.offset, [[0, P], [1, 2]])
            ).then_inc(setup_sem, 16)
            for it in range(NPT):
                b = it % 2
                r = bass.ts(tid(it), P)
                sync.wait_ge(act_sem, A(it, 3))
                sync.dma_start(x0_out[r, :], o0[b][:]).then_inc(so0[b], 16)
                if b == 0:
                    sync.wait_ge(vec_sem, V(it, 5))
                    sync.dma_start(xc_out[r, :], oc[0][:]).then_inc(soc[0], 16)
            for sem_pair in (so0, so1, soc):
                sync.wait_ge(sem_pair[0], 16 * ((NPT + 1) // 2))
                if NPT > 1:
                    sync.wait_ge(sem_pair[1], 16 * (NPT // 2))

        @block.vector
        def _(vector):
            vector.wait_ge(setup_sem, 32)
            nc.vector.tensor_scalar_mul(nb[:], bb[:], -1.0).then_inc(vec_sem, 1)
            for it in range(NPT):
                b = it % 2
                xs = it % 3
                vector.wait_ge(inx[xs], x_done(it))
                if it >= 1:
                    vector.wait_ge(act_sem, A(it - 1, 1))  # prod0 free
                nc.vector.tensor_mul(prod0[:], xt[xs][:], w0v).then_inc(
                    vec_sem, 1
                )
                if it >= 1:
                    vector.wait_ge(act_sem, A(it - 1, 2))  # prod1 free
                nc.vector.tensor_mul(prod1[:], xt[xs][:], w1v).then_inc(
                    vec_sem, 1
                )
                vector.wait_ge(act_sem, A(it, 1))  # z0 = red0 done
                nc.vector.tensor_scalar(
                    out=m[:, 0:1],
                    in0=z[:, 0:1],
                    scalar1=nb[:, 0:1],
                    scalar2=None,
                    op0=Alu.is_gt,
                ).then_inc(vec_sem, 1)
                vector.wait_ge(act_sem, A(it, 2))  # z1 = red1 done
                nc.vector.tensor_scalar(
                    out=m[:, 1:2],
                    in0=z[:, 1:2],
                    scalar1=nb[:, 1:2],
                    scalar2=None,
                    op0=Alu.is_gt,
                ).then_inc(vec_sem, 1)
                vector.wait_ge(act_sem, A(it, 3))  # o0[b] written
                vector.wait_ge(pool_sem, it + 1)  # o1[b] written
                if it >= 2:
                    vector.wait_ge(soc[b], slot_free(it))  # oc[b] stored
                nc.vector.tensor_add(oc[b][:], o0[b][:], o1[b][:]).then_inc(
                    vec_sem, 1
                )

        @block.scalar
        def _(scalar):
            # x loads ride the Activation HWDGE ring so they never queue
            # behind store waits on the SP ring.
            for it in range(min(3, NPT)):
                scalar.dma_start(
                    xt[it % 3][:], x_in[bass.ts(tid(it), P), :]
                ).then_inc(inx[it % 3], 16)
            for it in range(NPT):
                b = it % 2
                xs = it % 3
                scalar.wait_ge(vec_sem, V(it, 1))  # mult0 done
                nc.scalar.activation(
                    prod0[:], prod0[:], Copy, accum_out=z[:, 0:1]
                ).then_inc(act_sem, 1)
                scalar.wait_ge(vec_sem, V(it, 2))  # mult1 done
                nc.scalar.activation(
                    prod1[:], prod1[:], Copy, accum_out=z[:, 1:2]
                ).then_inc(act_sem, 1)
                scalar.wait_ge(vec_sem, V(it, 3))  # m0 ready
                if it >= 2:
                    scalar.wait_ge(so0[b], slot_free(it))  # o0[b] stored
                nc.scalar.activation(
                    o0[b][:], xt[xs][:], Copy, scale=m[:, 0:1]
                ).then_inc(act_sem, 1)
                j = it + 3
                if j < NPT:
                    # prefetch; slot j%3 == xs free once tile it's last
                    # x readers retired: mults (V(it,2) waited above),
                    # o0 (program order), o1 on Pool (explicit wait).
                    scalar.wait_ge(pool_sem, it + 1)
                    scalar.dma_start(
                        xt[xs][:], x_in[bass.ts(tid(j), P), :]
                    ).then_inc(inx[xs], 16)

        @block.gpsimd
        def _(gpsimd):
            # o1 = x * m1 on the Pool engine; x1 + odd-tile combined
            # stores ride the SWDGE (Pool) ring.
            for it in range(NPT):
                b = it % 2
                xs = it % 3
                r = bass.ts(tid(it), P)
                gpsimd.wait_ge(inx[xs], x_done(it))
                gpsimd.wait_ge(vec_sem, V(it, 4))  # m1 ready
                if it >= 2:
                    gpsimd.wait_ge(so1[b], slot_free(it))  # o1[b] stored
                nc.gpsimd.tensor_scalar_mul(
                    o1[b][:], xt[xs][:], m[:, 1:2]
                ).then_inc(pool_sem, 1)
                gpsimd.dma_start(x1_out[r, :], o1[b][:]).then_inc(so1[b], 16)
                if b == 1:
                    gpsimd.wait_ge(vec_sem, V(it, 5))  # oc ready
                    gpsimd.dma_start(xc_out[r, :], oc[1][:]).then_inc(
                        soc[1], 16
                    )

    nc.finalize()
    return nc


def _get_nc(n_pass=1):
    key = ("nc", n_pass)
    if key not in _CACHE:
        _CACHE[key] = _build(n_pass=n_pass)
    return _CACHE[key]


def _bf16_to_f32(a):
    """Exact bf16 -> f32 widening (16-bit left shift into the f32 layout)."""
    u = np.asarray(a).view(np.uint16).astype(np.uint32) << 16
    return u.view(np.float32)


def _get_runner(n_pass=1):
    """Build (once) a jitted 8-core shard_map runner for the bass module,
    mirroring bass2jax.run_bass_via_pjrt but cached across calls."""
    key = ("fn", n_pass)
    if key in _CACHE:
        return _CACHE[key]
    import jax
    from jax.sharding import Mesh, PartitionSpec
    from jax.experimental.shard_map import shard_map
    from concourse import bass2jax

    nc = _get_nc(n_pass)
    bass2jax.install_neuronx_cc_hook()
    partition_name = (
        nc.partition_id_tensor.name if nc.partition_id_tensor else None
    )
    in_names, out_names, out_avals = [], [], []
    for alloc in nc.m.functions[0].allocations:
        if not isinstance(alloc, mybir.MemoryLocationSet):
            continue
        name = alloc.memorylocations[0].name
        if alloc.kind == "ExternalInput":
            if name != partition_name:
                in_names.append(name)
        elif alloc.kind == "ExternalOutput":
            out_names.append(name)
            shape = tuple(alloc.tensor_shape)
            out_avals.append(
                jax.core.ShapedArray(shape, mybir.dt.np(alloc.dtype))
            )
    n_params = len(in_names)
    n_outs = len(out_avals)
    all_names = in_names + out_names
    if partition_name is not None:
        all_names.append(partition_name)
    donate = tuple(range(n_params, n_params + n_outs))

    def _body(*args):
        operands = list(args)
        if partition_name is not None:
            operands.append(bass2jax.partition_id_tensor())
        outs = bass2jax._bass_exec_p.bind(
            *operands,
            out_avals=tuple(out_avals),
            in_names=tuple(all_names),
            out_names=tuple(out_names),
            lowering_input_output_aliases=(),
            sim_require_finite=True,
            sim_require_nnan=True,
            nc=nc,
        )
        return tuple(outs)

    devices = jax.devices()[:N_CORES]
    mesh = Mesh(np.asarray(devices), ("core",))
    fn = jax.jit(
        shard_map(
            _body,
            mesh=mesh,
            in_specs=(PartitionSpec("core"),) * (n_params + n_outs),
            out_specs=(PartitionSpec("core"),) * n_outs,
            check_rep=False,
        ),
        donate_argnums=donate,
        keep_unused=True,
    )
    runner = (fn, in_names, out_names, out_avals)
    _CACHE[key] = runner
    return runner


def _run_fast(x, gate_w, gate_b, n_pass=1):
    """Execute via the cached jitted runner; returns (x0, x1, combined)."""
    fn, in_names, out_names, out_avals = _get_runner(n_pass)
    full = {"x": x, "gate_w": gate_w, "gate_b": gate_b}
    concat_in = []
    for nm in in_names:
        if nm == "x":
            concat_in.append(x)  # already [N, D]; shard_map splits axis 0
        else:
            a = full[nm]
            concat_in.append(np.concatenate([a] * N_CORES, axis=0))
    zeros = [
        np.zeros((N_CORES * av.shape[0], *av.shape[1:]), av.dtype)
        for av in out_avals
    ]
    outs = fn(*concat_in, *zeros)
    by_name = {nm: _bf16_to_f32(o) for nm, o in zip(out_names, outs)}
    return by_name["x0"], by_name["x1"], by_name["combined"]


def _run(x, gate_w, gate_b, trace=False, n_pass=1, **kw):
    x = np.ascontiguousarray(np.asarray(x, dtype=np.float32))
    gate_w = np.ascontiguousarray(np.asarray(gate_w, dtype=np.float32))
    gate_b = np.ascontiguousarray(np.asarray(gate_b, dtype=np.float32))
    assert x.shape == (N, D) and gate_w.shape == (D, 2) and gate_b.shape == (2,)

    nc = _get_nc(n_pass)
    in_maps = [
        {
            "x": x[c * SHARD : (c + 1) * SHARD],
            "gate_w": gate_w,
            "gate_b": gate_b,
        }
        for c in range(N_CORES)
    ]
    res = run_bass_kernel_spmd(
        nc, in_maps, core_ids=list(range(N_CORES)), trace=trace, **kw
    )
    x0 = np.concatenate(
        [_bf16_to_f32(res.results[c]["x0"]) for c in range(N_CORES)], axis=0
    )
    x1 = np.concatenate(
        [_bf16_to_f32(res.results[c]["x1"]) for c in range(N_CORES)], axis=0
    )
    xc = np.concatenate(
        [_bf16_to_f32(res.results[c]["combined"]) for c in range(N_CORES)],
        axis=0,
    )
    return (x0, x1, xc), res


def kernel(x, gate_w, gate_b):
    x = np.ascontiguousarray(np.asarray(x, dtype=np.float32))
    gate_w = np.ascontiguousarray(np.asarray(gate_w, dtype=np.float32))
    gate_b = np.ascontiguousarray(np.asarray(gate_b, dtype=np.float32))
    assert x.shape == (N, D) and gate_w.shape == (D, 2) and gate_b.shape == (2,)
    x0, x1, xc = _run_fast(x, gate_w, gate_b)
    return (x0, x1, xc)
